# revision 1
# baseline (speedup 1.0000x reference)
"""GAT forward (2-layer graph attention, B=4 N=2048 F=128 H=8 D=64 C=32)
as a Bass/Tile SPMD kernel on 8 Trainium2 NeuronCores.

Sharding: core c -> (batch b=c//2, query-row half c%2).  Each core computes
attention for its 1024 query rows over all 2048 keys for all 8 heads
(layer 1) and for the output head (layer 2).  The only cross-core exchange
is a 2-rank AllGather of the layer-2 projections [Wh2|g1|g2] ([1024,34] f32)
within each (2b, 2b+1) pair.

Key layout decision: attention logits are built TRANSPOSED,
e^T[j (keys) = partitions, i (queries) = free], so that
  - the PV matmul needs no operand transposes at all
    (lhsT = Wh[j,d] stationary, rhs = p[j,i] moving, out = h^T[d,i]),
  - softmax row sums come from a ones-column appended to Wh (PSUM row D).

Per 128x1024 logit tile the streaming work is exactly:
  DVE : one scalar_tensor_tensor  u = (mask_bias + f2[j]) + f1[i]
        (mask_bias in {0,-512} pre-baked host-side into bf16 tiles)
  ACT : Lrelu(u) ; Exp(z)
  PE  : 2 float32r matmuls accumulating h^T (with row-sum column)
Softmax division is deferred to the tiny h^T [64,1024] epilogue
(reciprocal via Ln/Exp of the row-sum), fused with ELU
(elu(v) = relu(v) + exp(min(v,0)) - 1, the -1 folded into a rank-1
correction matmul of the next layer's projection).
"""

import numpy as np
import ml_dtypes

import concourse.bass as bass
import concourse.tile as tile
from concourse import mybir
from concourse.bass_utils import run_bass_kernel_spmd

F32 = mybir.dt.float32
F32R = mybir.dt.float32r
BF16 = mybir.dt.bfloat16

B, N, F, H, D, C = 4, 2048, 128, 8, 64, 32
I = N // 2          # query rows per core
JT = N // 128       # key tiles
IC = I // 128       # query-row 128-chunks per core
KT = (H * D) // 128 # hidden-dim 128-chunks
ALPHA = 0.2
BIG = 512.0         # mask bias; exp(lrelu(-BIG+eps)) underflows to 0 in fp32
N_CORES = 8
REPLICA_GROUPS = [[0, 1], [2, 3], [4, 5], [6, 7]]

ADD = mybir.AluOpType.add
MAX = mybir.AluOpType.max
# NOTE: hardware "Lrelu" has a fixed 0.01 slope and ignores alpha;
# "Prelu" honors alpha (verified on HW) — it is the configurable leaky relu.
ACT_LRELU = mybir.ActivationFunctionType.Prelu
ACT_EXP = mybir.ActivationFunctionType.Exp
ACT_LN = mybir.ActivationFunctionType.Ln


def _split_multiwaits(nc):
    """Pinned walrus accepts only one sync-wait per instruction; Tile's exit
    drain (and occasionally others) carries several.  Hoist extras onto
    single-wait Drains on the same engine immediately before the owner."""
    n_fixed = 0
    for fn in nc.m.functions:
        for bb in fn.blocks:
            for name in [i.name for i in bb.instructions]:
                idx = [i.name for i in bb.instructions].index(name)
                inst = bb.instructions[idx]
                si = inst.sync_info
                if si is None or len(si.on_wait) <= 1:
                    continue
                waits = list(si.on_wait)
                for k, w in enumerate(waits[:-1]):
                    nd = mybir.InstDrain(
                        name=f"waitfix-{inst.name}-{k}", ins=[], outs=[])
                    nd.engine = inst.engine
                    nd.sync_info = mybir.SyncInfo(on_wait=[w], on_update=[])
                    nc.register_instruction(nd, overwrite=True)
                    bb.instructions.insert(idx + k, nd)
                inst.sync_info = mybir.SyncInfo(
                    on_wait=waits[-1:], on_update=list(si.on_update))
                n_fixed += 1
    return n_fixed


N_TILES = H * JT + JT        # 128 layer-1 logit tiles + 16 layer-2
N_PAIRS = N_TILES // 2       # lrelu/exp operate on pairs of tiles


def _spread(n, total, exclude=()):
    """n indices spread evenly over range(total) minus exclude."""
    avail = [t for t in range(total) if t not in exclude]
    if n >= len(avail):
        return set(avail)
    if n <= 0:
        return set()
    idx = np.linspace(0, len(avail) - 1, n).round().astype(int)
    return {avail[i] for i in idx}


def build_program(with_collective=True, cfg=None, repeat=1):
    """cfg routing knobs (engine load balancing across ACT/DVE/GPSIMD):
      gps_mask : #tile-PAIRS (of N_PAIRS) run in P_G mode: mask-add as
                 GPSIMD tensor_tensor + f2col via per-sub ACT Prelu bias
      dve_lrelu: #tile-PAIRS (of the rest) whose leaky-relu runs on DVE
      gps_ep   : route the per-head epilogue normalize-mul to GPSIMD
    """
    cfg = dict(cfg or {})
    gm = _spread(cfg.get("gps_mask", 0), N_PAIRS)
    gl = set()
    dl = _spread(cfg.get("dve_lrelu", 0), N_PAIRS, exclude=gm)
    route = (gm, gl, dl, bool(cfg.get("gps_ep", False)))

    nc = bass.Bass("TRN2", target_bir_lowering=False, debug=False,
                   enable_asserts=False, num_devices=N_CORES)

    xt_d = nc.dram_tensor("xt", [F, N], F32, kind="ExternalInput")
    xtl_d = nc.dram_tensor("xtl", [F, I], F32, kind="ExternalInput")
    mb_d = nc.dram_tensor("mb", [JT, 128, I], BF16, kind="ExternalInput")
    wext_d = nc.dram_tensor("wext", [H, F, D + 2], F32, kind="ExternalInput")
    a1rep_d = nc.dram_tensor("a1rep", [H, F, 128], F32, kind="ExternalInput")
    woext_d = nc.dram_tensor("woext", [KT, 128, C + 2], F32, kind="ExternalInput")
    wcorr_d = nc.dram_tensor("wcorr", [1, C + 2], F32, kind="ExternalInput")
    ident_d = nc.dram_tensor("ident", [128, 128], F32, kind="ExternalInput")
    outp_d = nc.dram_tensor("outp", [I, C], F32, kind="ExternalOutput")

    with tile.TileContext(nc) as tc:
        if repeat > 1:
            # timing rig: run the whole body `repeat` times on-device
            def body(iv, unroll=None):
                _build_body(nc, tc, xt_d, xtl_d, mb_d, wext_d, a1rep_d,
                            woext_d, wcorr_d, ident_d, outp_d,
                            with_collective, route)
            with tc.For_i(0, repeat, 1) as iv:
                body(iv)
        else:
            _build_body(nc, tc, xt_d, xtl_d, mb_d, wext_d, a1rep_d, woext_d,
                        wcorr_d, ident_d, outp_d, with_collective, route)
    _split_multiwaits(nc)
    return nc


def _logit_pair(nc, work, workp, pair_idx, route, tiles):
    """Two key-tiles' logits processed as one [128, 2, I] block, then ONE
    exp over the whole 2*I free dim (amortizes the per-op overhead).

    Modes per pair (GPSIMD supports no *Ptr opcodes, so per-partition-scalar
    STTs are DVE-only; GPS pairs instead fold f2col into the ACT Prelu bias):
      P_G (pair in gm): u[k] = mb + f1rep        (GPSIMD tensor_tensor)
                        z[k] = Prelu(u[k]+f2col) (ACT, per-sub bias)
      STD: u[k] = (mb + f2col) + f1rep           (DVE STT)
           z = Prelu(u) paired on ACT, or max(0.2u, u) STT on DVE (dl)
    """
    gm, gl, dl, _ = route
    u = work.tile([128, 2, I], F32, tag="u")
    if pair_idx in gm:
        for k, (t, mb_ap, f2col_ap, f1rep_ap) in enumerate(tiles):
            nc.gpsimd.tensor_add(u[:, k, :], mb_ap, f1rep_ap)
        for k, (t, mb_ap, f2col_ap, f1rep_ap) in enumerate(tiles):
            nc.scalar.activation(u[:, k, :], u[:, k, :], ACT_LRELU,
                                 bias=f2col_ap, alpha=ALPHA)
    else:
        for k, (t, mb_ap, f2col_ap, f1rep_ap) in enumerate(tiles):
            nc.vector.scalar_tensor_tensor(
                out=u[:, k, :], in0=mb_ap, scalar=f2col_ap, in1=f1rep_ap,
                op0=ADD, op1=ADD)
        if pair_idx in gl or pair_idx in dl:
            nc.vector.scalar_tensor_tensor(
                out=u[:], in0=u[:], scalar=ALPHA, in1=u[:],
                op0=mybir.AluOpType.mult, op1=MAX)
        else:
            nc.scalar.activation(u[:], u[:], ACT_LRELU, alpha=ALPHA)
    p = workp.tile([128, 2, I], F32R, tag="p")
    nc.scalar.activation(p[:], u[:], ACT_EXP)
    return p


def _build_body(nc, tc, xt_d, xtl_d, mb_d, wext_d, a1rep_d, woext_d,
                wcorr_d, ident_d, outp_d, with_collective, route):
    from contextlib import ExitStack
    gps_ep = route[3]
    ctx = ExitStack()
    with ctx:
        singles = ctx.enter_context(tc.tile_pool(name="singles", bufs=1))
        psA = ctx.enter_context(tc.tile_pool(name="psA", bufs=2, space="PSUM"))
        psB = ctx.enter_context(tc.tile_pool(name="psB", bufs=1, space="PSUM"))
        psC = ctx.enter_context(tc.tile_pool(name="psC", bufs=2, space="PSUM"))
        dram = ctx.enter_context(tc.tile_pool(name="dram", bufs=1, space="DRAM"))

        # ---------------- persistent loads ----------------
        # order = head-0 critical path first: the first logit pair needs
        # mb[0:2], f1rep (xtl+a1rep) and fcol[h0] (xt+wext, in phase 0)
        mb_s = singles.tile([128, JT, I], BF16)
        for jt in range(2):
            nc.sync.dma_start(out=mb_s[:, jt, :], in_=mb_d.ap()[jt])
        xtl_s = singles.tile([F, I], F32)
        nc.sync.dma_start(out=xtl_s[:], in_=xtl_d.ap())
        a1rep_s = singles.tile([F, H, 128], F32)
        nc.sync.dma_start(out=a1rep_s[:], in_=a1rep_d.ap().rearrange("h f e -> f h e"))
        for jt in range(2, JT):
            nc.sync.dma_start(out=mb_s[:, jt, :], in_=mb_d.ap()[jt])
        wcorr_s = singles.tile([1, C + 2], F32)
        nc.sync.dma_start(out=wcorr_s[:], in_=wcorr_d.ap())
        ident_s = singles.tile([128, 128], F32)
        nc.sync.dma_start(out=ident_s[:], in_=ident_d.ap())
        woext_s = singles.tile([128, KT, C + 2], F32R)

        ones_s = singles.tile([1, 128], F32)
        nc.gpsimd.memset(ones_s[:], 1.0)

        whbuf = singles.tile([128, H, JT, D + 1], F32R)
        nc.gpsimd.memset(whbuf[:, :, :, D:D + 1].bitcast(F32), 1.0)
        fcol = singles.tile([128, H, JT], F32)
        hcatT = singles.tile([128, KT, I], F32R)

        # ---------------- phase 0 (scoped; freed before work pools) ------
        # Wh tiles + f columns for ALL heads up front; only xt/wext live here.
        with tc.tile_pool(name="ph0", bufs=1) as ph0:
            xt_s = ph0.tile([F, N], F32)
            nc.sync.dma_start(out=xt_s[:], in_=xt_d.ap())
            wext_s = ph0.tile([F, H, D + 2], F32)
            nc.sync.dma_start(out=wext_s[:],
                              in_=wext_d.ap().rearrange("h f e -> f h e"))
            woext_raw = ph0.tile([128, KT, C + 2], F32)
            nc.sync.dma_start(out=woext_raw[:],
                              in_=woext_d.ap().rearrange("k f e -> f k e"))
            nc.any.tensor_copy(out=woext_s[:], in_=woext_raw[:])
            for h in range(H):
                for jt in range(JT):
                    whp = psA.tile([128, D + 2], F32, tag="small")
                    nc.tensor.matmul(whp[:],
                                     lhsT=xt_s[:, jt * 128:(jt + 1) * 128],
                                     rhs=wext_s[:, h, :])
                    nc.any.tensor_copy(out=whbuf[:, h, jt, 0:D],
                                       in_=whp[:, 0:D])
                    nc.any.tensor_copy(out=fcol[:, h, jt:jt + 1],
                                       in_=whp[:, D + 1:D + 2])

        work = ctx.enter_context(tc.tile_pool(name="work", bufs=4))
        workp = ctx.enter_context(tc.tile_pool(name="workp", bufs=3))
        ep1 = ctx.enter_context(tc.tile_pool(name="ep1", bufs=1))
        ep2 = ctx.enter_context(tc.tile_pool(name="ep2", bufs=2))

        # ---------------- layer 1 ----------------
        for h in range(H):
            # f1rep[p, i] = f1[i] (replicated over partitions):
            # lhsT = a1rep[h] (Wa1 in every column), rhs = xT local columns
            f1p = psB.tile([128, I], F32, tag="rep")
            for hf in range(I // 512):
                sl = slice(hf * 512, (hf + 1) * 512)
                nc.tensor.matmul(f1p[:, sl], lhsT=a1rep_s[:, h, :],
                                 rhs=xtl_s[:, sl])
            f1rep_s = ep2.tile([128, I], F32, tag="f1rep")
            nc.any.tensor_copy(out=f1rep_s[:], in_=f1p[:])

            # attention over e^T tiles [j=128, i=I]
            hT = psC.tile([D + 1, I], F32, tag="acc")
            for jp in range(JT // 2):
                tiles = []
                for k in range(2):
                    jt = jp * 2 + k
                    tiles.append((h * JT + jt, mb_s[:, jt, :],
                                  fcol[:, h, jt:jt + 1], f1rep_s[:]))
                p = _logit_pair(nc, work, workp, h * (JT // 2) + jp, route, tiles)
                for k in range(2):
                    jt = jp * 2 + k
                    for hf in range(I // 512):
                        sl = slice(hf * 512, (hf + 1) * 512)
                        nc.tensor.matmul(hT[:, sl],
                                         lhsT=whbuf[:, h, jt, :],
                                         rhs=p[:, k, sl],
                                         start=(jt == 0), stop=(jt == JT - 1))

            # epilogue: r = 1/S via Ln+Exp, v = hT*r, hcat_raw = elu(v)+1
            rh = ep1.tile([1, I], F32, tag="rh")
            nc.scalar.activation(rh[:], hT[D:D + 1, :], ACT_LN)
            nc.scalar.activation(rh[:], rh[:], ACT_EXP, scale=-1.0)
            hT_s = ep2.tile([D, I], F32, tag="hT")
            nc.any.tensor_copy(out=hT_s[:], in_=hT[0:D, :])
            rbcp = psC.tile([D, I], F32, tag="acc")
            for hf in range(I // 512):
                sl = slice(hf * 512, (hf + 1) * 512)
                nc.tensor.matmul(rbcp[:, sl], lhsT=ones_s[0:1, 0:D],
                                 rhs=rh[0:1, sl])
            v = ep1.tile([D, I], F32, tag="v")
            if gps_ep:
                rbc_s = ep1.tile([D, I], F32, tag="rbc")
                nc.any.tensor_copy(out=rbc_s[:], in_=rbcp[:])
                nc.gpsimd.tensor_mul(v[:], hT_s[:], rbc_s[:])
            else:
                nc.vector.tensor_mul(v[:], hT_s[:], rbcp[:])
            t = ep1.tile([D, I], F32, tag="t")
            nc.vector.tensor_scalar_min(t[:], v[:], 0.0)
            nc.scalar.activation(t[:], t[:], ACT_EXP)
            dst = hcatT[(h % 2) * D:(h % 2) * D + D, h // 2, :]
            nc.vector.scalar_tensor_tensor(
                out=dst, in0=v[:], scalar=0.0, in1=t[:], op0=MAX, op1=ADD)

        # ---------------- layer 2 projection + gather ----------------
        wh2loc = singles.tile([128, IC, C + 2], F32)
        gin = dram.tile([I, C + 2], F32)
        for ic in range(IC):
            w2p = psA.tile([128, C + 2], F32, tag="small")
            for kt in range(KT):
                nc.tensor.matmul(
                    w2p[:],
                    lhsT=hcatT[:, kt, ic * 128:(ic + 1) * 128],
                    rhs=woext_s[:, kt, :],
                    start=(kt == 0), stop=False)
            nc.tensor.matmul(w2p[:], lhsT=ones_s[0:1, :], rhs=wcorr_s[:],
                             start=False, stop=True)
            nc.any.tensor_copy(out=wh2loc[:, ic, :], in_=w2p[:])
            nc.sync.dma_start(out=gin[ic * 128:(ic + 1) * 128, :],
                              in_=wh2loc[:, ic, :])

        gout = dram.tile([N, C + 2], F32)
        if with_collective:
            nc.gpsimd.collective_compute(
                "AllGather", mybir.AluOpType.bypass,
                replica_groups=REPLICA_GROUPS,
                ins=[gin.opt()], outs=[gout.opt()])
        else:  # timing-model variant: fake the exchange with two local copies
            nc.sync.dma_start(out=gout[0:I, :], in_=gin[:])
            nc.sync.dma_start(out=gout[I:N, :], in_=gin[:])

        # g1rep[p,i] = g1[i]: transpose local g1 columns into a row, broadcast
        g1rowp = psB.tile([1, I], F32, tag="rep")
        for ic in range(IC):
            nc.tensor.transpose(g1rowp[0:1, ic * 128:(ic + 1) * 128],
                                in_=wh2loc[:, ic, 0:1], identity=ident_s[:])
        g1row_s = ep1.tile([1, I], F32, tag="g1row")
        nc.any.tensor_copy(out=g1row_s[:], in_=g1rowp[:])
        g1rp = psB.tile([128, I], F32, tag="rep")
        for hf in range(I // 512):
            sl = slice(hf * 512, (hf + 1) * 512)
            nc.tensor.matmul(g1rp[:, sl], lhsT=ones_s[0:1, :],
                             rhs=g1row_s[0:1, sl])
        g1rep_s = singles.tile([128, I], F32)
        nc.any.tensor_copy(out=g1rep_s[:], in_=g1rp[:])

        # gathered rows: [g1, g2, Wh2(32)] + ones column -> [128, 35] f32r
        # (DMA writes raw f32 bits; f32r here only tags the matmul operand)
        wh2gr = singles.tile([128, JT, C + 3], F32R)
        nc.gpsimd.memset(wh2gr[:, :, C + 2:C + 3].bitcast(F32), 1.0)
        for jt in range(JT):
            nc.sync.dma_start(out=wh2gr[:, jt, 0:C + 2].bitcast(F32),
                              in_=gout[jt * 128:(jt + 1) * 128, :])

        # ---------------- layer 2 attention ----------------
        o2T = psC.tile([C + 1, I], F32, tag="acc")
        for jp in range(JT // 2):
            tiles = []
            for k in range(2):
                jt = jp * 2 + k
                tiles.append((H * JT + jt, mb_s[:, jt, :],
                              wh2gr[:, jt, 1:2].bitcast(F32), g1rep_s[:]))
            p = _logit_pair(nc, work, workp, H * (JT // 2) + jp, route, tiles)
            for k in range(2):
                jt = jp * 2 + k
                for hf in range(I // 512):
                    sl = slice(hf * 512, (hf + 1) * 512)
                    nc.tensor.matmul(o2T[:, sl],
                                     lhsT=wh2gr[:, jt, 2:C + 3],
                                     rhs=p[:, k, sl],
                                     start=(jt == 0), stop=(jt == JT - 1))

        # ---------------- finalize ----------------
        r2 = ep1.tile([1, I], F32, tag="r2")
        nc.scalar.activation(r2[:], o2T[C:C + 1, :], ACT_LN)
        nc.scalar.activation(r2[:], r2[:], ACT_EXP, scale=-1.0)
        rbc2p = psC.tile([C, I], F32, tag="acc")
        for hf in range(I // 512):
            sl = slice(hf * 512, (hf + 1) * 512)
            nc.tensor.matmul(rbc2p[:, sl], lhsT=ones_s[0:1, 0:C],
                             rhs=r2[0:1, sl])
        rbc2_s = ep1.tile([C, I], F32, tag="rbc2")
        nc.any.tensor_copy(out=rbc2_s[:], in_=rbc2p[:])
        oT_s = ep1.tile([C, I], F32, tag="oT")
        nc.vector.tensor_mul(oT_s[:], o2T[0:C, :], rbc2_s[:])
        for k in range(IC):
            ofp = psA.tile([128, C], F32, tag="small")
            nc.tensor.transpose(ofp[:], in_=oT_s[:, k * 128:(k + 1) * 128],
                                identity=ident_s[0:C, 0:C])
            ofs = ep2.tile([128, C], F32, tag="ofs")
            nc.any.tensor_copy(out=ofs[:], in_=ofp[:])
            nc.sync.dma_start(out=outp_d.ap()[k * 128:(k + 1) * 128, :],
                              in_=ofs[:])


# --------------------------------------------------------------------------
# host side
# --------------------------------------------------------------------------

def shard_inputs(x, adj, W, a1, a2, Wo, ao1, ao2):
    x = np.asarray(x, np.float32)
    adj = np.asarray(adj)
    W = np.asarray(W, np.float32)
    a1 = np.asarray(a1, np.float32)
    a2 = np.asarray(a2, np.float32)
    Wo = np.asarray(Wo, np.float32)
    ao1 = np.asarray(ao1, np.float32)
    ao2 = np.asarray(ao2, np.float32)

    wvec1 = np.einsum("hfd,hd->hf", W, a1)          # [H, F]
    wvec2 = np.einsum("hfd,hd->hf", W, a2)
    wext = np.concatenate([W, wvec1[:, :, None], wvec2[:, :, None]],
                          axis=2).astype(np.float32)
    a1rep = np.repeat(wvec1[:, :, None], 128, axis=2).astype(np.float32)
    wo1 = Wo @ ao1                                   # [512]
    wo2 = Wo @ ao2
    woflat = np.concatenate([wo1[:, None], wo2[:, None], Wo], 1)  # [512, 34]
    woext = woflat.reshape(KT, 128, C + 2).astype(np.float32)
    wcorr = (-woflat.sum(0))[None, :].astype(np.float32)
    ident = np.eye(128, dtype=np.float32)

    in_maps = []
    for c in range(N_CORES):
        b, half = c // 2, c % 2
        i0 = half * I
        xt = np.ascontiguousarray(x[b].T)            # [F, N]
        xtl = np.ascontiguousarray(xt[:, i0:i0 + I])
        adjt = adj[b, i0:i0 + I, :].T                # [N, I] = (j, i)
        mb = np.where(adjt > 0, np.float32(0.0), np.float32(-BIG))
        mb = np.ascontiguousarray(mb.reshape(JT, 128, I)).astype(
            ml_dtypes.bfloat16)
        in_maps.append({
            "xt": xt, "xtl": xtl, "mb": mb, "wext": wext,
            "a1rep": a1rep, "woext": woext, "wcorr": wcorr, "ident": ident,
        })
    return in_maps


# Engine routing chosen by cost-model sweep (TimelineSim) and verified on
# hardware: ACT/DVE/GPSIMD busy ~260/238/124 us, modeled e2e ~366 us/core.
DEFAULT_CFG = {"gps_mask": 28, "dve_lrelu": 38, "gps_ep": True}

_CACHE = {}


def _program():
    if "nc" not in _CACHE:
        _CACHE["nc"] = build_program(with_collective=True, cfg=DEFAULT_CFG)
    return _CACHE["nc"]


def kernel(**inputs):
    nc = _program()
    in_maps = shard_inputs(**inputs)
    res = run_bass_kernel_spmd(nc, in_maps, list(range(N_CORES)))
    _CACHE["last_results"] = res
    out = np.empty((B, N, C), np.float32)
    for c in range(N_CORES):
        b, half = c // 2, c % 2
        out[b, half * I:(half + 1) * I, :] = res.results[c]["outp"]
    return out



# revision 48
# speedup vs baseline: 2.1614x; 2.1614x over previous
"""GAT forward (2-layer graph attention, B=4 N=2048 F=128 H=8 D=64 C=32)
as a Bass/Tile SPMD kernel on 8 Trainium2 NeuronCores.

Sharding: core c -> (batch b=c//2, query-row half c%2).  Each core computes
attention for its 1024 query rows over all 2048 keys for all 8 heads
(layer 1) and for the output head (layer 2).  The only cross-core exchange
is a 2-rank AllGather of the layer-2 projections [0.8g1|0.8g2|Wh2]
([1024,34] fp16) within each (2b, 2b+1) pair.

Key algebraic reformulation (vs. the naive mask-add/lrelu/exp pipeline):

  exp(lrelu(u)) = exp(0.2u) * max(exp(0.8u), 1)        (exact, exp monotone)

with u = f1[i] + f2[j] rank-1.  The exp(0.2*f1[i]) factor is constant in j
and cancels between softmax numerator and denominator, so per logit tile
(e^T layout: j=partitions, i=free) only TWO cheap DVE passes remain:

  TS : t[j,i] = max(E8rep[i] * E2c[j], e2c[j])          (tensor_scalar, 4x fp16)
       where E8rep = exp(0.8 f1) replicated, E2c = exp(f2), e2c = exp(0.2 f2)
  TT : p[j,i] = t[j,i] * adj[j,i]                       (tensor_tensor, 2x fp16)

t = exp(0.2 f2) * max(exp(0.8(f1+f2)), 1), so sum_j p equals the true
softmax denominator / exp(0.2 f1) and the PV matmul (lhsT=[Wh|1], rhs=p)
accumulates numerator and denominator with the same cancelled factor.
No per-logit transcendentals, no mask bias add.  The 0.8 scales for
f1/f2/g1/g2 are baked into the weight columns host-side.

Per-head epilogue: r = 1/S (DVE reciprocal), rbc = partition-broadcast on
GPSIMD, v = hT*rbc, elu via relu(v) + exp(min(v,0)) with the -1 folded into
a rank-1 correction matmul of the layer-2 projection (wcorr).
"""

import numpy as np

import concourse.bass as bass
import concourse.tile as tile
from concourse import mybir
from concourse.bass_utils import run_bass_kernel_spmd

F32 = mybir.dt.float32
FP16 = mybir.dt.float16

B, N, F, H, D, C = 4, 2048, 128, 8, 64, 32
I = N // 2          # query rows per core
JT = N // 128       # key tiles
IC = I // 128       # query-row 128-chunks per core
KT = (H * D) // 128 # hidden-dim 128-chunks
N_CORES = 8
REPLICA_GROUPS = [[0, 1], [2, 3], [4, 5], [6, 7]]

ADD = mybir.AluOpType.add
MAX = mybir.AluOpType.max
MIN = mybir.AluOpType.min
MULT = mybir.AluOpType.mult
ACT_EXP = mybir.ActivationFunctionType.Exp
ACT_LN = mybir.ActivationFunctionType.Ln


def _split_multiwaits(nc):
    """Pinned walrus accepts only one sync-wait per instruction; Tile's exit
    drain (and occasionally others) carries several.  Hoist extras onto
    single-wait Drains on the same engine immediately before the owner."""
    n_fixed = 0
    for fn in nc.m.functions:
        for bb in fn.blocks:
            for name in [i.name for i in bb.instructions]:
                idx = [i.name for i in bb.instructions].index(name)
                inst = bb.instructions[idx]
                si = inst.sync_info
                if si is None or len(si.on_wait) <= 1:
                    continue
                waits = list(si.on_wait)
                for k, w in enumerate(waits[:-1]):
                    nd = mybir.InstDrain(
                        name=f"waitfix-{inst.name}-{k}", ins=[], outs=[])
                    nd.engine = inst.engine
                    nd.sync_info = mybir.SyncInfo(on_wait=[w], on_update=[])
                    nc.register_instruction(nd, overwrite=True)
                    bb.instructions.insert(idx + k, nd)
                inst.sync_info = mybir.SyncInfo(
                    on_wait=waits[-1:], on_update=list(si.on_update))
                n_fixed += 1
    return n_fixed


def _spread(n, total, exclude=()):
    """n indices spread evenly over range(total) minus exclude."""
    avail = [t for t in range(total) if t not in exclude]
    if n >= len(avail):
        return set(avail)
    if n <= 0:
        return set()
    idx = np.linspace(0, len(avail) - 1, n).round().astype(int)
    return {avail[i] for i in idx}


def build_program(with_collective=True, cfg=None, repeat=1):
    """cfg knobs (engine load balancing):
      grp      : jt tiles per mask group (2/4/8)
      gps_mask : #mask groups (of H*JT/grp + JT/grp) whose mask TT runs on
                 GPSIMD instead of DVE
      gps_vmul : #heads (of 9, incl layer-2 final) whose epilogue normalize
                 mult runs on GPSIMD instead of DVE
      act_recip: compute 1/S via ACT Ln+Exp instead of DVE reciprocal
      ph0_eng  : cycle of engines for phase-0 PSUM->SBUF copies
    """
    cfg = dict(cfg or {})
    grp = int(cfg.get("grp", 4))
    route = {
        "grp": grp,
        "gps_pat": tuple(cfg.get("gps_pat", (1, 2))),
        "gps_vmul": _spread(cfg.get("gps_vmul", 0), H + 1),
        "act_recip": _spread(cfg.get("act_recip", 0), H + 1),
        "ph0_eng": cfg.get("ph0_eng", ("scalar",)),
        "work_bufs": int(cfg.get("work_bufs", 4)),
    }

    nc = bass.Bass("TRN2", target_bir_lowering=False, debug=False,
                   enable_asserts=False, num_devices=N_CORES)

    xt_d = nc.dram_tensor("xt", [F, N], FP16, kind="ExternalInput")
    xtl_d = nc.dram_tensor("xtl", [F, I], FP16, kind="ExternalInput")
    adjt_d = nc.dram_tensor("adjt", [JT, 128, I], FP16, kind="ExternalInput")
    wext_d = nc.dram_tensor("wext", [H, F, D + 1], FP16, kind="ExternalInput")
    a1rep_d = nc.dram_tensor("a1rep", [H, F, 128], FP16, kind="ExternalInput")
    woext_d = nc.dram_tensor("woext", [KT, 128, C + 2], FP16,
                             kind="ExternalInput")
    wcorr_d = nc.dram_tensor("wcorr", [1, C + 2], FP16, kind="ExternalInput")
    ident_d = nc.dram_tensor("ident", [128, 128], FP16, kind="ExternalInput")
    outp_d = nc.dram_tensor("outp", [I, C], F32, kind="ExternalOutput")

    with tile.TileContext(nc) as tc:
        if repeat > 1:
            def body(iv, unroll=None):
                _build_body(nc, tc, xt_d, xtl_d, adjt_d, wext_d, a1rep_d,
                            woext_d, wcorr_d, ident_d, outp_d,
                            with_collective, route)
            with tc.For_i(0, repeat, 1) as iv:
                body(iv)
        else:
            _build_body(nc, tc, xt_d, xtl_d, adjt_d, wext_d, a1rep_d,
                        woext_d, wcorr_d, ident_d, outp_d,
                        with_collective, route)
    _split_multiwaits(nc)
    return nc


def _build_body(nc, tc, xt_d, xtl_d, adjt_d, wext_d, a1rep_d, woext_d,
                wcorr_d, ident_d, outp_d, with_collective, route):
    from contextlib import ExitStack
    grp = route["grp"]
    gps_pat = route["gps_pat"]
    gps_vmul = route["gps_vmul"]
    act_recip = route["act_recip"]
    ph0_eng = route["ph0_eng"]
    gidx = [0]  # running mask-group counter (cycles gps_pat)

    ctx = ExitStack()
    with ctx:
        singles = ctx.enter_context(tc.tile_pool(name="singles", bufs=1))
        psA = ctx.enter_context(tc.tile_pool(name="psA", bufs=2, space="PSUM"))
        psB = ctx.enter_context(tc.tile_pool(name="psB", bufs=1, space="PSUM"))
        psC = ctx.enter_context(tc.tile_pool(name="psC", bufs=2, space="PSUM"))
        dram = ctx.enter_context(tc.tile_pool(name="dram", bufs=1, space="DRAM"))

        # ---------------- persistent loads ----------------
        # order: phase-0 critical path (xtl for E8, xt/wext for Wh) first,
        # then adj in 2-jt batches (few SP issues, parallel DMA queues)
        xtl_s = singles.tile([F, I], FP16)
        nc.sync.dma_start(out=xtl_s[:], in_=xtl_d.ap())
        a1rep_s = singles.tile([F, H, 128], FP16)
        nc.sync.dma_start(out=a1rep_s[:],
                          in_=a1rep_d.ap().rearrange("h f e -> f h e"))
        adj_s = singles.tile([128, JT, I], FP16)

        def load_adj():
            for j0 in range(0, JT, 2):
                nc.sync.dma_start(
                    out=adj_s[:, j0:j0 + 2, :],
                    in_=adjt_d.ap()[j0:j0 + 2].rearrange("j p i -> p j i"))
            wcorr_s = singles.tile([1, C + 2], FP16)
            nc.sync.dma_start(out=wcorr_s[:], in_=wcorr_d.ap())
            ident_s = singles.tile([128, 128], FP16)
            nc.sync.dma_start(out=ident_s[:], in_=ident_d.ap())
            woext_s = singles.tile([128, KT, C + 2], FP16)
            nc.sync.dma_start(out=woext_s[:],
                              in_=woext_d.ap().rearrange("k f e -> f k e"))
            return wcorr_s, ident_s, woext_s

        ones_s = singles.tile([1, 128], FP16)
        nc.gpsimd.memset(ones_s[:], 1.0)

        # whbuf: [Wh (D cols) | ones]; col D transiently holds 0.8*f2
        whbuf = singles.tile([128, H, JT, D + 1], FP16)
        c8 = singles.tile([128, H, JT], F32)    # 0.8*f2 per (h, jt-row)
        E2c = singles.tile([128, H, JT], F32)   # exp(f2)
        e2c = singles.tile([128, H, JT], F32)   # exp(0.2*f2)
        hcatT = singles.tile([128, KT, I], FP16)

        work = ctx.enter_context(
            tc.tile_pool(name="work", bufs=route["work_bufs"]))
        repp = ctx.enter_context(tc.tile_pool(name="repp", bufs=2))
        ep1 = ctx.enter_context(tc.tile_pool(name="ep1", bufs=2))

        def emit_e8(h):
            """f1 replicate + exp for head h, in 512-col PSUM chunks (psA)
            so it can overlap the previous head's attention."""
            E8 = repp.tile([128, I], FP16, tag="e8")
            for hf in range(I // 512):
                sl = slice(hf * 512, (hf + 1) * 512)
                fp = psA.tile([128, 512], F32, tag="small")
                nc.tensor.matmul(fp[:], lhsT=a1rep_s[:, h, :],
                                 rhs=xtl_s[:, sl])
                nc.scalar.activation(E8[:, sl], fp[:], ACT_EXP)
            return E8

        e8_tiles = {0: emit_e8(0)}

        # ---------------- phase 0: Wh for all heads ----------------
        with tc.tile_pool(name="ph0", bufs=1) as ph0:
            xt_s = ph0.tile([F, N], FP16)
            nc.sync.dma_start(out=xt_s[:], in_=xt_d.ap())
            wext_s = ph0.tile([F, H, D + 1], FP16)
            nc.sync.dma_start(out=wext_s[:],
                              in_=wext_d.ap().rearrange("h f e -> f h e"))
            wcorr_s, ident_s, woext_s = load_adj()
            HG = 4  # heads per matmul group; hg-outer so head 0 unblocks early
            ci = 0
            for hg in range(H // HG):
                hsl = slice(hg * HG, (hg + 1) * HG)
                for q in range(JT // 4):
                    for jt in range(q * 4, q * 4 + 4):
                        whp = psA.tile([128, HG, D + 1], F32, tag="small")
                        nc.tensor.matmul(
                            whp[:], lhsT=xt_s[:, jt * 128:(jt + 1) * 128],
                            rhs=wext_s[:, hsl, :])
                        eng = getattr(nc, ph0_eng[ci % len(ph0_eng)])
                        ci += 1
                        if hasattr(eng, "tensor_copy"):
                            eng.tensor_copy(out=whbuf[:, hsl, jt, :],
                                            in_=whp[:])
                        else:
                            eng.copy(out=whbuf[:, hsl, jt, :], in_=whp[:])
                    # extract 0.8*f2 cols, overwrite with the ones column;
                    # per-quad so head 0 can start while phase 0 runs
                    qsl = slice(q * 4, q * 4 + 4)
                    nc.vector.tensor_copy(out=c8[:, hsl, qsl],
                                          in_=whbuf[:, hsl, qsl, D])
                    nc.gpsimd.memset(whbuf[:, hsl, qsl, D:D + 1], 1.0)
                    nc.scalar.activation(E2c[:, hsl, qsl], c8[:, hsl, qsl],
                                         ACT_EXP, scale=1.25)
                    nc.scalar.activation(e2c[:, hsl, qsl], c8[:, hsl, qsl],
                                         ACT_EXP, scale=0.25)

        def attention(hT, rep, E2cv, e2cv, wh_lhsT, nrows, hooks=None,
                      g_range=None, pat=None):
            """one attention pass: logits+mask+PV over JT key tiles.
            rep: exp(0.8*f1)-style replicated [128, I] fp16
            E2cv/e2cv: jt -> per-partition scalar AP [128, 1]
            wh_lhsT(jt) -> AP of [128, nrows] PV weights
            hooks: {group_idx: fn} emitted after that group (pipelining)
            pat: override the gps subtile pattern for this call"""
            hooks = hooks or {}
            gs = list(g_range if g_range is not None else range(JT // grp))
            for gi, g in enumerate(gs):
                if pat is not None:
                    ng = pat[gi % len(pat)]
                else:
                    ng = gps_pat[gidx[0] % len(gps_pat)]
                    gidx[0] += 1
                kd = grp - ng
                tq = work.tile([128, grp, I], FP16, tag="t")
                for k in range(grp):
                    jt = g * grp + k
                    nc.vector.tensor_scalar(
                        out=tq[:, k, :], in0=rep[:],
                        scalar1=E2cv(jt), scalar2=e2cv(jt),
                        op0=MULT, op1=MAX)
                # mask into separate tiles per engine (no WAW between the
                # DVE and GPSIMD halves): DVE takes subtiles [0:kd),
                # GPSIMD takes [kd:grp)
                asl = adj_s[:, g * grp:(g + 1) * grp, :]
                pg = None
                if ng:
                    pg = work.tile([128, max(gps_pat), I], FP16, tag="pg")
                    nc.gpsimd.tensor_tensor(
                        out=pg[:, 0:ng, :], in0=tq[:, kd:grp, :],
                        in1=asl[:, kd:grp, :], op=MULT)
                pd = work.tile([128, grp, I], FP16, tag="pd")
                if kd:
                    nc.vector.tensor_tensor(
                        out=pd[:, 0:kd, :], in0=tq[:, 0:kd, :],
                        in1=asl[:, 0:kd, :], op=MULT)
                for k in range(grp):
                    jt = g * grp + k
                    for hf in range(I // 512):
                        sl = slice(hf * 512, (hf + 1) * 512)
                        rhs_ap = (pd[:, k, sl] if k < kd
                                  else pg[:, k - kd, sl])
                        nc.tensor.matmul(hT[:, sl], lhsT=wh_lhsT(jt),
                                         rhs=rhs_ap,
                                         start=(jt == 0), stop=(jt == JT - 1))
                if g in hooks:
                    hooks[g]()

        def normalize_a(hT, nrows, ep_idx):
            """1/S row + partition-replicate (ACT+PE only, no DVE ops)."""
            rS = ep1.tile([1, I], FP16, tag="r")
            if ep_idx in act_recip:
                rl = ep1.tile([1, I], F32, tag="rl")
                nc.scalar.activation(rl[:], hT[nrows:nrows + 1, :], ACT_LN)
                nc.scalar.activation(rS[:], rl[:], ACT_EXP, scale=-1.0)
            else:
                rf = ep1.tile([1, I], F32, tag="rf")
                nc.vector.reciprocal(out=rf[:], in_=hT[nrows:nrows + 1, :])
                nc.vector.tensor_copy(out=rS[:], in_=rf[:])
            rbcp = psB.tile([nrows, I], F32, tag="rep")
            for hf in range(I // 512):
                sl = slice(hf * 512, (hf + 1) * 512)
                nc.tensor.matmul(rbcp[:, sl], lhsT=ones_s[0:1, 0:nrows],
                                 rhs=rS[0:1, sl])
            rbc = ep1.tile([nrows, I], F32, tag="rbc")
            nc.scalar.copy(out=rbc[:], in_=rbcp[:])
            return rbc

        def normalize_b(hT, rbc, nrows, ep_idx, out_dtype=FP16):
            """v = hT[0:nrows] * rbc"""
            v = ep1.tile([nrows, I], out_dtype, tag="v")
            if ep_idx in gps_vmul:
                # GPSIMD is SBUF-only: stage hT through SBUF via ACT
                h_s = ep1.tile([nrows, I], F32, tag="hs")
                nc.scalar.copy(out=h_s[:], in_=hT[0:nrows, :])
                nc.gpsimd.tensor_mul(v[:], h_s[:], rbc[:])
            else:
                nc.vector.tensor_tensor(out=v[:], in0=hT[0:nrows, :],
                                        in1=rbc[:], op=MULT)
            return v

        def normalize(hT, nrows, ep_idx, out_dtype=FP16):
            return normalize_b(hT, normalize_a(hT, nrows, ep_idx), nrows,
                               ep_idx, out_dtype)

        # ---------------- layer 1 ----------------
        # Epilogue staged over the NEXT head's groups so its cross-engine
        # chain never head-of-line-blocks the DVE mask stream:
        #   stage A (g0): 1/S + replicate (ACT/PE)    stage B (g2): v, elu
        #   stage C (g3): hcat combine
        st = {}

        def epi_a(h, hT):
            st[h] = (hT, normalize_a(hT, D, h))

        def epi_b(h):
            hT, rbc = st[h]
            v = normalize_b(hT, rbc, D, h)
            t = ep1.tile([D, I], FP16, tag="t")
            nc.vector.tensor_scalar(out=t[:], in0=v[:], scalar1=0.0,
                                    scalar2=None, op0=MIN)
            nc.scalar.activation(t[:], t[:], ACT_EXP)
            rv = ep1.tile([D, I], FP16, tag="rv")
            nc.vector.tensor_scalar(out=rv[:], in0=v[:], scalar1=0.0,
                                    scalar2=None, op0=MAX)
            st[h] = (t, rv)

        def epi_c(h):
            t, rv = st.pop(h)
            dst = hcatT[(h % 2) * D:(h % 2) * D + D, h // 2, :]
            nc.vector.tensor_tensor(out=dst, in0=rv[:], in1=t[:], op=ADD)

        pend = {}  # h -> hT awaiting epilogue (deferred into next head)
        for h in range(H):
            E8rep = e8_tiles.pop(h)
            hT = psC.tile([D + 1, I], F32, tag="acc")
            hooks = {}
            prev = pend.pop(h - 1, None)
            hooks[0] = (lambda h=h, prev=prev: (
                e8_tiles.update({h + 1: emit_e8(h + 1)}) if h + 1 < H
                else None,
                epi_a(h - 1, prev) if prev is not None else None))
            if prev is not None:
                hooks[2] = (lambda h=h: epi_b(h - 1))
                hooks[3] = (lambda h=h: epi_c(h - 1))
            attention(hT, E8rep,
                      lambda jt, h=h: E2c[:, h, jt:jt + 1],
                      lambda jt, h=h: e2c[:, h, jt:jt + 1],
                      lambda jt, h=h: whbuf[:, h, jt, :], D + 1,
                      hooks=hooks)
            pend[h] = hT
        hT7 = pend.pop(H - 1)
        epi_a(H - 1, hT7)
        epi_b(H - 1)
        epi_c(H - 1)

        # ---------------- layer 2 projection + exchange ----------------
        # Host key order is [own half | peer half], so the local 8 key
        # tiles need no communication: layer-2 attention over them starts
        # while the pair exchange (AllReduce-sum; peer = sum - own) is in
        # flight, hiding the collective entirely.
        wh2ls = singles.tile([128, IC, C + 2], F32)   # local proj, f32
        g1c = singles.tile([128, IC], FP16)           # 0.8*g1 columns
        g1rowp = psB.tile([1, I], FP16, tag="rep")
        gin = [dram.tile([I // 2, C + 2], F32, name=f"gin{h2}")
               for h2 in range(2)]
        gsum = [dram.tile([I // 2, C + 2], F32, name=f"gsum{h2}")
                for h2 in range(2)]
        for ic in range(IC):
            w2p = psA.tile([128, C + 2], F32, tag="small")
            for kt in range(KT):
                nc.tensor.matmul(
                    w2p[:],
                    lhsT=hcatT[:, kt, ic * 128:(ic + 1) * 128],
                    rhs=woext_s[:, kt, :],
                    start=(kt == 0), stop=False)
            nc.tensor.matmul(w2p[:], lhsT=ones_s[0:1, :], rhs=wcorr_s[:],
                             start=False, stop=True)
            nc.scalar.copy(out=wh2ls[:, ic, :], in_=w2p[:])
            nc.scalar.copy(out=g1c[:, ic:ic + 1], in_=w2p[:, 0:1])
            nc.tensor.transpose(g1rowp[0:1, ic * 128:(ic + 1) * 128],
                                in_=g1c[:, ic:ic + 1], identity=ident_s[:])
            if ic % 4 == 3:  # exchange in halves to overlap with compute
                h2 = ic // 4
                nc.sync.dma_start(
                    out=gin[h2][:].rearrange("(ic p) c -> p ic c", p=128),
                    in_=wh2ls[:, h2 * 4:(h2 + 1) * 4, :])
                if with_collective:
                    nc.gpsimd.collective_compute(
                        "AllReduce", mybir.AluOpType.add,
                        replica_groups=REPLICA_GROUPS,
                        ins=[gin[h2].opt()], outs=[gsum[h2].opt()])
                else:  # timing-model variant: fake the exchange
                    nc.sync.dma_start(out=gsum[h2][:], in_=gin[h2][:])

        # local fp16 tiles [0.8g1, 0.8g2, Wh2 | ones] + scalar columns
        wh2l = singles.tile([128, IC, C + 3], FP16)
        nc.gpsimd.memset(wh2l[:, :, C + 2:C + 3], 1.0)
        nc.vector.tensor_copy(out=wh2l[:, :, 0:C + 2], in_=wh2ls[:])
        c82l = singles.tile([128, IC], F32)
        nc.vector.tensor_copy(out=c82l[:], in_=wh2ls[:, :, 1])
        E2c2l = singles.tile([128, IC], F32)
        e2c2l = singles.tile([128, IC], F32)
        nc.scalar.activation(E2c2l[:], c82l[:], ACT_EXP, scale=1.25)
        nc.scalar.activation(e2c2l[:], c82l[:], ACT_EXP, scale=0.25)

        # G8rep[p,i] = exp(0.8*g1[i]) replicated: g1 columns transposed
        # per-ic above, broadcast via ones-matmul, exp.
        g1row_s = ep1.tile([1, I], FP16, tag="g1row")
        nc.scalar.copy(out=g1row_s[:], in_=g1rowp[:])
        G8rep = repp.tile([128, I], FP16, tag="e8")
        for hf in range(I // 512):
            sl = slice(hf * 512, (hf + 1) * 512)
            g8p = psA.tile([128, 512], F32, tag="small")
            nc.tensor.matmul(g8p[:], lhsT=ones_s[0:1, :],
                             rhs=g1row_s[0:1, sl])
            nc.scalar.activation(G8rep[:, sl], g8p[:], ACT_EXP)

        # ---------------- layer 2 attention (local tiles first) --------
        o2T = psC.tile([C + 1, I], F32, tag="acc")
        wh2rs = singles.tile([128, IC, C + 2], F32)
        wh2r = singles.tile([128, IC, C + 3], FP16)
        nc.gpsimd.memset(wh2r[:, :, C + 2:C + 3], 1.0)
        c82r = singles.tile([128, IC], F32)
        E2c2r = singles.tile([128, IC], F32)
        e2c2r = singles.tile([128, IC], F32)

        def l2_lhsT(jt):
            if jt < IC:
                return wh2l[:, jt, 2:C + 3]
            return wh2r[:, jt - IC, 2:C + 3]

        def l2_E2(jt):
            if jt < IC:
                return E2c2l[:, jt:jt + 1]
            return E2c2r[:, jt - IC:jt - IC + 1]

        def l2_e2(jt):
            if jt < IC:
                return e2c2l[:, jt:jt + 1]
            return e2c2r[:, jt - IC:jt - IC + 1]

        def unpack_half(h2):
            """peer rows for half h2: wh2r = gsum - wh2ls, then scalars"""
            hsl = slice(h2 * 4, (h2 + 1) * 4)
            nc.sync.dma_start(
                out=wh2rs[:, hsl, :],
                in_=gsum[h2][:].rearrange("(jt p) c -> p jt c", p=128))
            nc.vector.tensor_tensor(out=wh2r[:, hsl, 0:C + 2],
                                    in0=wh2rs[:, hsl, :],
                                    in1=wh2ls[:, hsl, :],
                                    op=mybir.AluOpType.subtract)
            nc.vector.tensor_copy(out=c82r[:, hsl], in_=wh2r[:, hsl, 1])
            nc.scalar.activation(E2c2r[:, hsl], c82r[:, hsl], ACT_EXP,
                                 scale=1.25)
            nc.scalar.activation(e2c2r[:, hsl], c82r[:, hsl], ACT_EXP,
                                 scale=0.25)

        n_loc = IC // grp
        unpack_half(0)
        attention(o2T, G8rep, l2_E2, l2_e2, l2_lhsT, C + 1,
                  g_range=range(n_loc),
                  hooks={0: lambda: unpack_half(1)})
        # keep GPSIMD's bigger bites off the final groups' critical tail
        attention(o2T, G8rep, l2_E2, l2_e2, l2_lhsT, C + 1,
                  g_range=range(n_loc, JT // grp), pat=(2, 1))

        oT = normalize(o2T, C, H, out_dtype=FP16)
        ofp = psA.tile([128, IC, C], FP16, tag="small")
        for k in range(IC):
            nc.tensor.transpose(ofp[:, k, :], in_=oT[:, k * 128:(k + 1) * 128],
                                identity=ident_s[0:C, 0:C])
        ofs = ep1.tile([128, IC, C], F32, tag="ofs")
        nc.scalar.copy(out=ofs[:], in_=ofp[:])
        nc.sync.dma_start(out=outp_d.ap().rearrange("(ic p) c -> p ic c",
                                                    p=128),
                          in_=ofs[:])


# --------------------------------------------------------------------------
# host side
# --------------------------------------------------------------------------

def shard_inputs(x, adj, W, a1, a2, Wo, ao1, ao2):
    x = np.asarray(x, np.float32)
    adj = np.asarray(adj)
    W = np.asarray(W, np.float32)
    a1 = np.asarray(a1, np.float32)
    a2 = np.asarray(a2, np.float32)
    Wo = np.asarray(Wo, np.float32)
    ao1 = np.asarray(ao1, np.float32)
    ao2 = np.asarray(ao2, np.float32)

    wvec1 = np.einsum("hfd,hd->hf", W, a1)          # [H, F] f1 generator
    wvec2 = np.einsum("hfd,hd->hf", W, a2)          # [H, F] f2 generator
    wext = np.concatenate([W, 0.8 * wvec2[:, :, None]],
                          axis=2).astype(np.float16)          # [H, F, D+1]
    a1rep = np.repeat(0.8 * wvec1[:, :, None], 128, axis=2).astype(np.float16)
    wo1 = Wo @ ao1                                   # [512]
    wo2 = Wo @ ao2
    woflat = np.concatenate([0.8 * wo1[:, None], 0.8 * wo2[:, None], Wo], 1)
    woext = woflat.reshape(KT, 128, C + 2).astype(np.float16)
    wcorr = (-woflat.sum(0))[None, :].astype(np.float16)
    ident = np.eye(128, dtype=np.float16)

    in_maps = []
    for c in range(N_CORES):
        b, half = c // 2, c % 2
        i0 = half * I
        # key order per core: [own half | peer half] so the program's
        # "local" key tiles are always jt 0..IC-1 (core-agnostic)
        perm = np.r_[i0:i0 + I, (I - i0):(I - i0) + I] % N
        xt = np.ascontiguousarray(x[b].T[:, perm]).astype(np.float16)
        xtl = np.ascontiguousarray(xt[:, 0:I])
        adjt = (adj[b, i0:i0 + I, :][:, perm].T > 0).astype(np.float16)
        adjt = np.ascontiguousarray(adjt.reshape(JT, 128, I))
        in_maps.append({
            "xt": xt, "xtl": xtl, "adjt": adjt, "wext": wext,
            "a1rep": a1rep, "woext": woext, "wcorr": wcorr, "ident": ident,
        })
    return in_maps


DEFAULT_CFG = {"grp": 4, "gps_pat": (1, 2), "gps_vmul": 0, "act_recip": 9,
               "ph0_eng": ("scalar", "vector"), "work_bufs": 4}

_CACHE = {}


def _program():
    if "nc" not in _CACHE:
        _CACHE["nc"] = build_program(with_collective=True, cfg=DEFAULT_CFG)
    return _CACHE["nc"]


def kernel(**inputs):
    nc = _program()
    in_maps = shard_inputs(**inputs)
    res = run_bass_kernel_spmd(nc, in_maps, list(range(N_CORES)))
    _CACHE["last_results"] = res
    out = np.empty((B, N, C), np.float32)
    for c in range(N_CORES):
        b, half = c // 2, c % 2
        out[b, half * I:(half + 1) * I, :] = res.results[c]["outp"]
    return out
